# revision 5
# baseline (speedup 1.0000x reference)
"""Bass/Tile kernel for KeyFrameAttention on 8 NeuronCores (TRN2).

Math (per batch item b):
    q = x @ Wq + bq ; k = x @ Wk + bk ; v = x @ Wv + bv
    scores[n,m] = q[n]·k[m];  masked-fill(Mask==0, -1e20); softmax over m of scores/sqrt(C)
    att_feat[n,c] = sum_m v[m,c] * attn[m,n]          (attention applied TRANSPOSED)
    out = att_feat @ Wr + br

Sharding: data-parallel over batch B=64 -> 8 batch items per core.

Host-side prep inside kernel(): cast x / Mask / W* to bf16 (validated: end-to-end
rel err ~5e-3 vs fp32 reference thanks to fp32 PSUM accumulation).

Per-core plan (bf16 matmuls, fp32 PSUM accumulation):
  xT  [C,N]   via PE transposes of x tiles (contraction needs c on partitions)
  qT,kT [C,N] = W.T @ x.T   (lhsT = W tile, rhs = xT)      -> bf16 SBUF
  v   [N,C]   natural       (lhsT = xT tile, rhs = Wv)     -> bf16 SBUF
  scores tile [128n, 512m] = qT.T @ kT ; masked softmax via the (+BIG)*mask trick:
      t = (scores + BIG)*mask ; e = exp(s*t - s*max(t)) ; masked -> exp(-s*max) == 0
  att_featT [C,N]: lhsT = v tile, rhs = attn tile (no attn transpose needed)
  out [N,C]:  lhsT = afT tile, rhs = Wr ; + br ; DMA out.

Weights are streamed from HBM per batch item (SBUF can't hold 4x CxC + working set).
"""

import math

import numpy as np

B, N, C = 64, 512, 1280
NCORES = 8
BPC = B // NCORES  # batch items per core
P = 128
NT = N // P  # 4  n-tiles
CT = C // P  # 10 c-tiles
BIG = 10000.0
SCALE = 1.0 / math.sqrt(float(C))
CF_SLICES = [(0, 512), (512, 512), (1024, 256)]  # free-dim chunks of C

_CACHE = {}


def _build_nc():
    import concourse.bass as bass
    import concourse.mybir as mybir
    import concourse.tile as tile
    from concourse import bacc
    from concourse.masks import make_identity

    f32 = mybir.dt.float32
    bf16 = mybir.dt.bfloat16
    AF = mybir.ActivationFunctionType
    ALU = mybir.AluOpType

    # Bacc (not raw Bass): its finalize() runs move_matmul_waits_to_ldweights +
    # generate_event_semaphores, which split multi-sem waits that otherwise
    # exceed the per-instruction ISA wait-slot limit in walrus codegen.
    nc = bacc.Bacc(None, target_bir_lowering=False)
    x_h = nc.declare_dram_parameter("x", [BPC, N, C], bf16, isOutput=False)
    m_h = nc.declare_dram_parameter("mask", [BPC, N, N], bf16, isOutput=False)
    wq_h = nc.declare_dram_parameter("wq", [C, C], bf16, isOutput=False)
    bq_h = nc.declare_dram_parameter("bq", [C], f32, isOutput=False)
    wk_h = nc.declare_dram_parameter("wk", [C, C], bf16, isOutput=False)
    bk_h = nc.declare_dram_parameter("bk", [C], f32, isOutput=False)
    wv_h = nc.declare_dram_parameter("wv", [C, C], bf16, isOutput=False)
    bv_h = nc.declare_dram_parameter("bv", [C], f32, isOutput=False)
    wr_h = nc.declare_dram_parameter("wr", [C, C], bf16, isOutput=False)
    br_h = nc.declare_dram_parameter("br", [C], f32, isOutput=False)
    out_h = nc.declare_dram_parameter("out", [BPC, N, C], f32, isOutput=True)

    def bcast_ap(handle):
        ap0 = handle[:]
        return bass.AP(tensor=ap0.tensor, offset=ap0.offset, ap=[[0, P], ap0.ap[0]])

    with tile.TileContext(nc) as tc:
        with (
            tc.tile_pool(name="sb", bufs=1) as sb,
            tc.tile_pool(name="ps", bufs=1, space="PSUM") as ps,
        ):
            # ---- biases (one-time)
            bq_sb = sb.tile([P, CT], f32, tag="bq", bufs=1, name="bq_sb")
            nc.sync.dma_start(out=bq_sb, in_=bq_h[:].rearrange("(co p) -> p co", p=P))
            bk_sb = sb.tile([P, CT], f32, tag="bk", bufs=1, name="bk_sb")
            nc.sync.dma_start(out=bk_sb, in_=bk_h[:].rearrange("(co p) -> p co", p=P))
            bv_sb = sb.tile([P, C], f32, tag="bv", bufs=1, name="bv_sb")
            nc.sync.dma_start(out=bv_sb, in_=bcast_ap(bv_h))
            br_sb = sb.tile([P, C], f32, tag="br", bufs=1, name="br_sb")
            nc.sync.dma_start(out=br_sb, in_=bcast_ap(br_h))

            ident = sb.tile([P, P], bf16, tag="ident", bufs=1, name="ident")
            make_identity(nc, ident)

            for b in range(BPC):
                # ---- Phase A: load x (bf16), transpose -> xT
                xbf = []
                for nt in range(NT):
                    xb = sb.tile([P, C], bf16, tag="xbf", bufs=8, name=f"xb{b}_{nt}")
                    nc.sync.dma_start(out=xb, in_=x_h[b, nt * P : (nt + 1) * P, :])
                    xbf.append(xb)
                xT = []
                for ct in range(CT):
                    pt = ps.tile([P, N], bf16, tag="tr", bufs=2, name=f"pt{b}_{ct}")
                    for nt in range(NT):
                        nc.tensor.transpose(
                            pt[:, nt * P : (nt + 1) * P],
                            xbf[nt][:, ct * P : (ct + 1) * P],
                            ident,
                        )
                    xt = sb.tile([P, N], bf16, tag="xT", bufs=12, name=f"xt{b}_{ct}")
                    nc.scalar.copy(out=xt, in_=pt)
                    xT.append(xt)

                # ---- Phase B: qT, kT (lhsT = W tile), v (lhsT = xT tile)
                qT, kT = [], []
                for wh, dst, bias, wtag in (
                    (wq_h, qT, bq_sb, "q"),
                    (wk_h, kT, bk_sb, "k"),
                ):
                    wt = []
                    for ki in range(CT):
                        w = sb.tile(
                            [P, C], bf16, tag="w", bufs=12, name=f"w{b}_{wtag}_{ki}"
                        )
                        nc.sync.dma_start(out=w, in_=wh[ki * P : (ki + 1) * P, :])
                        wt.append(w)
                    for co in range(CT):
                        pm = ps.tile(
                            [P, N], f32, tag="mm", bufs=4, name=f"pq{b}_{wtag}_{co}"
                        )
                        for ki in range(CT):
                            nc.tensor.matmul(
                                pm,
                                wt[ki][:, co * P : (co + 1) * P],
                                xT[ki],
                                start=(ki == 0),
                                stop=(ki == CT - 1),
                            )
                        sbt = sb.tile(
                            [P, N], bf16, tag="qkT", bufs=20, name=f"qk{b}_{wtag}_{co}"
                        )
                        nc.vector.tensor_scalar_add(
                            out=sbt, in0=pm, scalar1=bias[:, co : co + 1]
                        )
                        dst.append(sbt)

                wv_t = []
                for ki in range(CT):
                    w = sb.tile([P, C], bf16, tag="w", bufs=12, name=f"w{b}_v_{ki}")
                    nc.sync.dma_start(out=w, in_=wv_h[ki * P : (ki + 1) * P, :])
                    wv_t.append(w)
                v_sb = []
                for mt in range(NT):
                    vt = sb.tile([P, C], bf16, tag="v", bufs=5, name=f"v{b}_{mt}")
                    for cf0, cfw in CF_SLICES:
                        pm = ps.tile(
                            [P, cfw], f32, tag="mm", bufs=4, name=f"pv{b}_{mt}_{cf0}"
                        )
                        for ki in range(CT):
                            nc.tensor.matmul(
                                pm,
                                xT[ki][:, mt * P : (mt + 1) * P],
                                wv_t[ki][:, cf0 : cf0 + cfw],
                                start=(ki == 0),
                                stop=(ki == CT - 1),
                            )
                        nc.vector.tensor_tensor(
                            vt[:, cf0 : cf0 + cfw],
                            pm,
                            bv_sb[:, cf0 : cf0 + cfw],
                            ALU.add,
                        )
                    v_sb.append(vt)

                # ---- Phase C: scores + masked softmax per n-tile
                attn = []
                for it in range(NT):
                    pm = ps.tile([P, N], f32, tag="mm", bufs=4, name=f"psc{b}_{it}")
                    for ki in range(CT):
                        nc.tensor.matmul(
                            pm,
                            qT[ki][:, it * P : (it + 1) * P],
                            kT[ki],
                            start=(ki == 0),
                            stop=(ki == CT - 1),
                        )
                    mf = sb.tile([P, N], bf16, tag="mf", bufs=3, name=f"mf{b}_{it}")
                    nc.sync.dma_start(out=mf, in_=m_h[b, it * P : (it + 1) * P, :])
                    t = sb.tile([P, N], f32, tag="t", bufs=2, name=f"t{b}_{it}")
                    nc.vector.scalar_tensor_tensor(
                        out=t, in0=pm, scalar=BIG, in1=mf, op0=ALU.add, op1=ALU.mult
                    )
                    mx = sb.tile([P, 1], f32, tag="mx", bufs=2, name=f"mx{b}_{it}")
                    nc.vector.tensor_reduce(
                        out=mx, in_=t, axis=mybir.AxisListType.X, op=ALU.max
                    )
                    bias_ap = sb.tile([P, 1], f32, tag="bias", bufs=2, name=f"ba{b}_{it}")
                    nc.vector.tensor_scalar_mul(out=bias_ap, in0=mx, scalar1=-SCALE)
                    e = sb.tile([P, N], f32, tag="e", bufs=2, name=f"e{b}_{it}")
                    rs = sb.tile([P, 1], f32, tag="rs", bufs=2, name=f"rs{b}_{it}")
                    nc.scalar.activation(
                        out=e, in_=t, func=AF.Exp, bias=bias_ap, scale=SCALE, accum_out=rs
                    )
                    r = sb.tile([P, 1], f32, tag="r", bufs=2, name=f"r{b}_{it}")
                    nc.vector.reciprocal(out=r, in_=rs)
                    at = sb.tile([P, N], bf16, tag="attn", bufs=5, name=f"at{b}_{it}")
                    nc.vector.tensor_scalar_mul(out=at, in0=e, scalar1=r)
                    attn.append(at)

                # ---- Phase E: att_featT[c,n] = sum_m v[m,c] * attn[m,n]
                afT = []
                for co in range(CT):
                    pm = ps.tile([P, N], f32, tag="mm", bufs=4, name=f"pa{b}_{co}")
                    for mt in range(NT):
                        nc.tensor.matmul(
                            pm,
                            v_sb[mt][:, co * P : (co + 1) * P],
                            attn[mt],
                            start=(mt == 0),
                            stop=(mt == NT - 1),
                        )
                    af = sb.tile([P, N], bf16, tag="afT", bufs=11, name=f"af{b}_{co}")
                    nc.vector.tensor_copy(out=af, in_=pm)
                    afT.append(af)

                # ---- Phase F: out = att_feat @ Wr + br
                wr_t = []
                for ki in range(CT):
                    w = sb.tile([P, C], bf16, tag="w", bufs=12, name=f"w{b}_r_{ki}")
                    nc.sync.dma_start(out=w, in_=wr_h[ki * P : (ki + 1) * P, :])
                    wr_t.append(w)
                for it in range(NT):
                    osb = sb.tile([P, C], f32, tag="osb", bufs=2, name=f"o{b}_{it}")
                    for cf0, cfw in CF_SLICES:
                        pm = ps.tile(
                            [P, cfw], f32, tag="mm", bufs=4, name=f"po{b}_{it}_{cf0}"
                        )
                        for co in range(CT):
                            nc.tensor.matmul(
                                pm,
                                afT[co][:, it * P : (it + 1) * P],
                                wr_t[co][:, cf0 : cf0 + cfw],
                                start=(co == 0),
                                stop=(co == CT - 1),
                            )
                        nc.vector.tensor_tensor(
                            osb[:, cf0 : cf0 + cfw],
                            pm,
                            br_sb[:, cf0 : cf0 + cfw],
                            ALU.add,
                        )
                    nc.sync.dma_start(
                        out=out_h[b, it * P : (it + 1) * P, :], in_=osb
                    )
    nc.finalize()
    return nc


def _get_nc():
    if "nc" not in _CACHE:
        _CACHE["nc"] = _build_nc()
    return _CACHE["nc"]


def _run(inputs, trace=False):
    import ml_dtypes
    from concourse import bass_utils

    bf = ml_dtypes.bfloat16
    nc = _get_nc()
    x = np.ascontiguousarray(inputs["x"]).astype(bf)
    mask = np.ascontiguousarray(inputs["Mask"]).astype(bf)
    shared = {
        "wq": np.ascontiguousarray(inputs["Wq"]).astype(bf),
        "bq": np.ascontiguousarray(inputs["bq"], dtype=np.float32),
        "wk": np.ascontiguousarray(inputs["Wk"]).astype(bf),
        "bk": np.ascontiguousarray(inputs["bk"], dtype=np.float32),
        "wv": np.ascontiguousarray(inputs["Wv"]).astype(bf),
        "bv": np.ascontiguousarray(inputs["bv"], dtype=np.float32),
        "wr": np.ascontiguousarray(inputs["Wr"]).astype(bf),
        "br": np.ascontiguousarray(inputs["br"], dtype=np.float32),
    }
    in_maps = [
        {"x": x[c * BPC : (c + 1) * BPC], "mask": mask[c * BPC : (c + 1) * BPC], **shared}
        for c in range(NCORES)
    ]
    res = bass_utils.run_bass_kernel_spmd(
        nc, in_maps, core_ids=list(range(NCORES)), trace=trace
    )
    out = np.concatenate([r["out"] for r in res.results], axis=0)
    return out, res


def kernel(**inputs):
    out, _ = _run(inputs)
    return out
